# revision 66
# baseline (speedup 1.0000x reference)
"""Trainium2 Bass kernel for MeanAggregator GNN message passing.

Computation (see reference):
  h = tanh(BN_trainmode(features @ W.T + b)) ; out = row-mean over sampled
  neighbor set (deduped membership mask) of h rows.  The linear bias b
  cancels exactly inside train-mode BN (shift-invariant), so it is dropped.

Strategy (8 cores, SPMD), rev11 — gather-free, stats-decoupled tail:
  - Shard OUTPUT rows across cores (512 rows/core).  The host pre-gathers
    the feature rows for each (row, slot) entry: every output row gets
    S=18 slot-planes (17 samples + pad; pad slots carry weight 0); each
    core receives a dense [256, 9216] fp16 entry matrix, a [128, 9216]
    replicated weight matrix, and a [256, 512] weighted-mean feature
    matrix (xbar).
  - BN batch stats need the full table; only channel sums/sumsq are used,
    so the table shard + W ride in float8e4 packed per k-tile and each
    512-column chunk is a single DoubleRow matmul.  The CC AllGather of
    the [128,2] partials has a hard floor in this environment: the CC
    cores wake ~60-110us after kernel launch regardless of doorbell.
  - To keep that floor off the compute path, the tanh pipeline runs
    DURING the collective using per-core LOCAL shard stats (a0, b0), and
    the global stats enter only through an exact first-order correction:
      out = out0 + da*(mbar - Q1) + db*(1 - Q0)
    with out0 = sum_s w*y0, Q0 = sum_s w*y0^2, Q1 = sum_s w*y0^2*mm,
    mbar = W @ xbar (exact, since sum_s w*mm is linear), da = a - a0,
    db = b - b0.  Local stats are off by <2.5%, so the dropped
    second-order term contributes ~3e-4 end-to-end (measured).
    Post-collective work collapses to the scale/shift chain + four
    fused per-partition DVE ops + one output DMA.
  - Entries are laid out SLOT-MAJOR (entry (r,s) at column s*R+r), so
    each 512-entry chunk is one slot plane [E,R]: ACT drains y0 (and the
    raw mm copy) straight from PSUM; DVE forms the three products and
    folds each plane into flat fp16 accumulators — no strided segmented
    reductions and no work left on the collective's critical path except
    the per-partition scale chain and three fused correction ops.
  - Output is [128, 512] (channels x rows) fp16 per core; host
    transposes and converts.
"""

import sys

for _p in ("/opt/trn_rl_repo", "/root/.axon_site/_ro/trn_rl_repo"):
    if _p not in sys.path:
        sys.path.append(_p)

import ml_dtypes
import numpy as np

import concourse.bass as bass
import concourse.bacc as bacc
import concourse.tile as tile
import concourse.mybir as mybir
from concourse.bass_utils import run_bass_kernel_spmd

F32 = mybir.dt.float32
F16 = mybir.dt.float16
F8 = mybir.dt.float8e4
AF = mybir.ActivationFunctionType
OP = mybir.AluOpType
AX = mybir.AxisListType
PM = mybir.MatmulPerfMode

N_CORES = 8
U, F, E, B = 50000, 256, 128, 4096
S = 18                  # slot stride per output row (17 samples + 1 pad, even for DVE 2x)
UL = 6272               # per-core table rows for stats (49 * 128)
AW = E + UL             # fp8 pack width per k-tile: [W | table]
R = B // N_CORES        # 512 output rows per core
EN = R * S              # 8704 entries per core (= 17 * 512 exactly)
CH = 512                # entry / table chunk width (one PSUM bank)
BN_EPS = 1e-5

U_CHUNKS = [(i * CH, CH) for i in range(UL // CH)]
if UL % CH:
    U_CHUNKS.append((UL - UL % CH, UL % CH))
E_CHUNKS = [(i * CH, CH) for i in range(EN // CH)]
XA_PIECES = [(0, E + 1536), (E + 1536, 1536), (E + 3072, 1536),
             (E + 4608, 1664)]

_CACHE = {}
LAST_RESULTS = None
TRACE = False


def _build():
    if "nc" in _CACHE:
        return _CACHE["nc"]

    nc = bacc.Bacc("TRN2", target_bir_lowering=False, debug=False,
                   enable_asserts=False, num_devices=N_CORES)

    # ---- I/O ----
    xA = nc.dram_tensor("xA", [128, 2 * AW], F8, kind="ExternalInput")
    xgT = nc.dram_tensor("xgT", [F, EN], F16, kind="ExternalInput")
    xbT = nc.dram_tensor("xbT", [F, R], F16, kind="ExternalInput")
    Wt = nc.dram_tensor("Wt", [F, E], F16, kind="ExternalInput")
    gb = nc.dram_tensor("gb", [E, 4], F32, kind="ExternalInput")
    wrow = nc.dram_tensor("wrow", [128, EN], F16, kind="ExternalInput")
    outT = nc.dram_tensor("outT", [E, R], F16, kind="ExternalOutput")

    # ---- internal DRAM (stats exchange): ReduceScatter of my [128,2]
    # partial tiled 8x -> every rank's output shard IS the full sum ----
    ag_in = nc.dram_tensor("ag_in", [N_CORES * E, 2], F32)
    ag_out = nc.dram_tensor("ag_out", [E, 2], F32)

    RG = [list(range(N_CORES))]
    xA3 = xA.ap().rearrange("p (two m) -> p two m", two=2)

    with tile.TileContext(nc) as tc:
        with (
            tc.tile_pool(name="const", bufs=1) as cpool,
            tc.tile_pool(name="rot", bufs=3) as rot,
        ):
            # ---- stats-critical load first: fp8 [W | table] piece 0 ----
            xa = cpool.tile([128, 2, AW], F8, tag="xa")
            p0, pn = XA_PIECES[0]
            nc.sync.dma_start(xa[:, :, p0:p0 + pn], xA3[:, :, p0:p0 + pn])

            wt0 = cpool.tile([128, E], F16, tag="wt0")
            wt1 = cpool.tile([128, E], F16, tag="wt1")
            nc.sync.dma_start(wt0[:], Wt[0:128, :])
            nc.sync.dma_start(wt1[:], Wt[128:256, :])
            xb0 = cpool.tile([128, R], F16, tag="xb0")
            xb1 = cpool.tile([128, R], F16, tag="xb1")
            nc.sync.dma_start(xb0[:], xbT[0:128, :])
            nc.sync.dma_start(xb1[:], xbT[128:256, :])
            gbt = cpool.tile([E, 4], F32, tag="gbt")
            nc.sync.dma_start(gbt[:], gb[:])
            epscol = cpool.tile([E, 1], F32, tag="epscol")
            nc.vector.memset(epscol[:], BN_EPS)

            # remaining table pieces, then the entry tensors (the latter
            # land inside the CC window; the quiesce barrier of the
            # collective covers only DMAs issued before it, so the entry
            # loads are issued after the collective below)
            for p0, pn in XA_PIECES[1:]:
                nc.sync.dma_start(xa[:, :, p0:p0 + pn], xA3[:, :, p0:p0 + pn])

            n_ch = len(U_CHUNKS)
            musum = cpool.tile([E, n_ch], F32, tag="musum")
            ssq = cpool.tile([E, n_ch], F32, tag="ssq")
            mbar = cpool.tile([E, R], F32, tag="mbar")

            # ---- phase A: fp8 DoubleRow table GEMM -> sum / sumsq;
            # then the exact linear-aggregate GEMM mbar = W @ xbar ----
            with tc.tile_pool(name="psA", bufs=1, space="PSUM") as psA:
                for ci, (u0, un) in enumerate(U_CHUNKS):
                    ps = psA.tile([128, un], F32, tag=f"ps{ci % 4}")
                    nc.tensor.matmul(
                        ps[:], xa[:, :, 0:E], xa[:, :, E + u0:E + u0 + un],
                        start=True, stop=True, perf_mode=PM.DoubleRow)
                    nc.vector.tensor_reduce(musum[:, ci:ci + 1], ps[:],
                                            axis=AX.X, op=OP.add)
                    sqd = rot.tile([128, un], F16, tag="sqd")
                    nc.scalar.activation(sqd[:], ps[:], AF.Square,
                                         accum_out=ssq[:, ci:ci + 1])
                psm = psA.tile([128, R], F32, tag="psm")
                nc.tensor.matmul(psm[:], wt0[:], xb0[:],
                                 start=True, stop=False)
                nc.tensor.matmul(psm[:], wt1[:], xb1[:],
                                 start=False, stop=True)
                nc.vector.tensor_copy(mbar[:], psm[:])

            # ---- my stats partial (for the collective + local stats) ----
            stats_sb = cpool.tile([E, 2], F32, tag="stats_sb")
            nc.vector.tensor_reduce(stats_sb[:, 0:1], musum[:], axis=AX.X,
                                    op=OP.add)
            nc.vector.tensor_reduce(stats_sb[:, 1:2], ssq[:], axis=AX.X,
                                    op=OP.add)
            for k in range(N_CORES):
                nc.scalar.dma_start(ag_in[k * E:(k + 1) * E, :],
                                    stats_sb[:])

            # ---- LOCAL shard stats -> a0, b0 (per-partition columns) ----
            invn = gbt[:, 2:3]
            mu0 = cpool.tile([E, 1], F32, tag="mu0")
            nc.vector.tensor_tensor(mu0[:], stats_sb[:, 0:1], invn,
                                    op=OP.mult)
            ex20 = cpool.tile([E, 1], F32, tag="ex20")
            nc.vector.tensor_tensor(ex20[:], stats_sb[:, 1:2], invn,
                                    op=OP.mult)
            var0 = cpool.tile([E, 1], F32, tag="var0")
            musq0 = cpool.tile([E, 1], F32, tag="musq0")
            nc.vector.tensor_tensor(musq0[:], mu0[:], mu0[:], op=OP.mult)
            nc.vector.tensor_tensor(var0[:], ex20[:], musq0[:],
                                    op=OP.subtract)
            sd0 = cpool.tile([E, 1], F32, tag="sd0")
            nc.scalar.activation(sd0[:], var0[:], AF.Sqrt,
                                 bias=epscol[:, 0:1])
            rinv0 = cpool.tile([E, 1], F32, tag="rinv0")
            nc.vector.reciprocal(rinv0[:], sd0[:])
            a0 = cpool.tile([E, 1], F32, tag="a0")
            nc.vector.tensor_tensor(a0[:], rinv0[:], gbt[:, 0:1], op=OP.mult)
            msc0 = cpool.tile([E, 1], F32, tag="msc0")
            nc.vector.tensor_tensor(msc0[:], mu0[:], a0[:], op=OP.mult)
            b0 = cpool.tile([E, 1], F32, tag="b0")
            nc.vector.tensor_tensor(b0[:], gbt[:, 1:2], msc0[:],
                                    op=OP.subtract)

            # entry features + replicated weights interleaved in quarter
            # pieces so GEMM chunk 0 (needs BOTH xg halves) and the first
            # yw product (needs wmt) start as soon as possible; all
            # stream during the CC window
            wmt = cpool.tile([128, EN], F16, tag="wmt")
            xg0 = cpool.tile([128, EN], F16, tag="xg0")
            xg1 = cpool.tile([128, EN], F16, tag="xg1")
            PW = EN // 4
            for p in range(4):
                sl = slice(p * PW, (p + 1) * PW)
                nc.sync.dma_start(xg0[:, sl], xgT[0:128, sl])
                nc.sync.dma_start(xg1[:, sl], xgT[128:256, sl])
                nc.sync.dma_start(wmt[:, sl], wrow[:, sl])

            # ---- phase B inside the CC window: per 512-entry chunk
            # GEMM -> { ACT y0 = tanh(a0*mm+b0), DVE raw copy }, then
            # GpSimd yw/y2w, DVE q, and per-64-row-block reduces ----
            # Entries are laid out SLOT-MAJOR (entry (r, s) at column
            # s*R + r), so each 512-entry chunk is exactly one slot
            # plane [E, R] and the three maps are flat fp16 accumulator
            # adds instead of strided segmented reductions.
            out0 = cpool.tile([E, R], F16, tag="out0")
            q0m = cpool.tile([E, R], F16, tag="q0m")
            q1m = cpool.tile([E, R], F16, tag="q1m")

            with tc.tile_pool(name="psB", bufs=1, space="PSUM") as psB:
                for ci, (e0, en) in enumerate(E_CHUNKS):
                    sl = slice(e0, e0 + en)
                    ps = psB.tile([128, en], F32, tag=f"pb{ci % 4}")
                    nc.tensor.matmul(ps[:], wt0[:], xg0[:, sl],
                                     start=True, stop=False)
                    nc.tensor.matmul(ps[:], wt1[:], xg1[:, sl],
                                     start=False, stop=True)
                    y0 = rot.tile([128, en], F16, tag="y0")
                    nc.scalar.activation(y0[:], ps[:], AF.Tanh,
                                         bias=b0[:, 0:1], scale=a0[:, 0:1])
                    mm = rot.tile([128, en], F16, tag="mm")
                    nc.scalar.copy(mm[:], ps[:])
                    if ci == 0:
                        yw, y2w, q = out0, q0m, q1m
                        nc.vector.tensor_tensor(yw[:], y0[:], wmt[:, sl],
                                                op=OP.mult)
                        nc.vector.tensor_tensor(y2w[:], y0[:], yw[:],
                                                op=OP.mult)
                        nc.vector.tensor_tensor(q[:], y2w[:], mm[:],
                                                op=OP.mult)
                    else:
                        yw = rot.tile([128, en], F16, tag="yw")
                        nc.vector.tensor_tensor(yw[:], y0[:], wmt[:, sl],
                                                op=OP.mult)
                        y2w = rot.tile([128, en], F16, tag="y2w")
                        nc.vector.tensor_tensor(y2w[:], y0[:], yw[:],
                                                op=OP.mult)
                        q = rot.tile([128, en], F16, tag="q")
                        nc.vector.tensor_tensor(q[:], y2w[:], mm[:],
                                                op=OP.mult)
                        with nc.allow_low_precision("fp16 plane accums"):
                            nc.vector.tensor_tensor(out0[:], out0[:], yw[:],
                                                    op=OP.add)
                            nc.vector.tensor_tensor(q0m[:], q0m[:], y2w[:],
                                                    op=OP.add)
                            nc.vector.tensor_tensor(q1m[:], q1m[:], q[:],
                                                    op=OP.add)
            # preload the SQRT table while the collective is in flight
            dum = cpool.tile([E, 1], F16, tag="dum")
            nc.scalar.activation(dum[:], epscol[:], AF.Sqrt)

            # ---- the collective (issued after the gpsimd products so
            # its completion drain does not block them) ----
            nc.gpsimd.collective_compute(
                "ReduceScatter", OP.add, replica_groups=RG,
                ins=[ag_in.ap()], outs=[ag_out.ap()])

            # ---- CC result (already summed by CCE) -> correction ----
            # The scheduler's sim models the collective as near-instant
            # and would order the CC-gated chain BEFORE the last plane
            # accumulates on the vector queue, dragging them behind the
            # real collective.  A dummy copy q1m->stats_g pins the recv
            # DMA (WAW) and everything after it behind the accumulators.
            stats_g = cpool.tile([E, 2], F32, tag="stats_g")
            nc.vector.tensor_copy(stats_g[0:1, 0:1], q1m[0:1, R - 1:R])
            nc.sync.dma_start(stats_g[:], ag_out.ap())

            mu = cpool.tile([E, 1], F32, tag="mu")
            nc.vector.tensor_scalar_mul(mu[:], stats_g[:, 0:1], 1.0 / U)
            ex2 = cpool.tile([E, 1], F32, tag="ex2")
            nc.vector.tensor_scalar_mul(ex2[:], stats_g[:, 1:2], 1.0 / U)
            # varneg = mu^2 - ex2 = -var; sd = sqrt(-varneg + eps)
            varneg = cpool.tile([E, 1], F32, tag="varneg")
            nc.vector.scalar_tensor_tensor(varneg[:], mu[:], mu[:, 0:1],
                                           ex2[:], op0=OP.mult,
                                           op1=OP.subtract)
            sd = cpool.tile([E, 1], F32, tag="sd")
            nc.scalar.activation(sd[:], varneg[:], AF.Sqrt,
                                 bias=epscol[:, 0:1], scale=-1.0)
            rinv = cpool.tile([E, 1], F32, tag="rinv")
            nc.vector.reciprocal(rinv[:], sd[:])
            a_g = cpool.tile([E, 1], F32, tag="a_g")
            nc.vector.tensor_tensor(a_g[:], rinv[:], gbt[:, 0:1], op=OP.mult)
            # bneg = mu*a - beta = -b_global (single fused op)
            bneg = cpool.tile([E, 1], F32, tag="bneg")
            nc.vector.scalar_tensor_tensor(bneg[:], mu[:], a_g[:, 0:1],
                                           gbt[:, 1:2], op0=OP.mult,
                                           op1=OP.subtract)

            m_ = cpool.tile([E, 1], F32, tag="m_")     # a0 - a = -da
            nc.vector.tensor_tensor(m_[:], a0[:], a_g[:], op=OP.subtract)
            db = cpool.tile([E, 1], F32, tag="db")     # b - b0
            nc.vector.scalar_tensor_tensor(db[:], bneg[:], -1.0, b0[:],
                                           op0=OP.mult, op1=OP.subtract)
            n_ = cpool.tile([E, 1], F32, tag="n_")     # -db = b0 + bneg
            nc.vector.tensor_tensor(n_[:], b0[:], bneg[:], op=OP.add)

            # out = out0 + da*(mbar - Q1) + db*(1 - Q0), via sign-carry:
            #   s1 = m*Q1 + out0 ; t1 = m*mbar - s1 ; t2 = n*Q0 - t1
            #   out = t2 + db   — split column-wise across DVE and the
            # (post-collective idle) GpSimd engine
            s1 = cpool.tile([E, R], F16, tag="s1")
            t1 = cpool.tile([E, R], F16, tag="t1")
            t2 = cpool.tile([E, R], F16, tag="t2")
            outsb = cpool.tile([E, R], F16, tag="outsb")
            nc.vector.scalar_tensor_tensor(s1[:], q1m[:], m_[:, 0:1],
                                           out0[:], op0=OP.mult, op1=OP.add)
            nc.vector.scalar_tensor_tensor(t1[:], mbar[:], m_[:, 0:1],
                                           s1[:], op0=OP.mult,
                                           op1=OP.subtract)
            nc.vector.scalar_tensor_tensor(t2[:], q0m[:], n_[:, 0:1],
                                           t1[:], op0=OP.mult,
                                           op1=OP.subtract)
            nc.vector.tensor_scalar_add(outsb[:], t2[:], db[:, 0:1])

            nc.sync.dma_start(outT.ap(), outsb[:])

    nc.compile()
    _CACHE["nc"] = nc
    return nc


def _prep_inputs(features, W, gamma, beta, row_idx, col_idx):
    """Host-side sharding: dedup mask entries, lay out 17 slots per output
    row (zero-weight padding), pre-gather entry feature rows per core."""
    features = np.asarray(features, dtype=np.float32)
    W = np.asarray(W, dtype=np.float32)
    gamma = np.asarray(gamma, dtype=np.float32)
    beta = np.asarray(beta, dtype=np.float32)
    row = np.asarray(row_idx).astype(np.int64)
    col = np.asarray(col_idx).astype(np.int64)

    # dedup (row, col) pairs: mask "set" semantics
    key = row * np.int64(U) + col
    order = np.argsort(key, kind="stable")
    sk = key[order]
    keep_s = np.ones(len(sk), dtype=bool)
    keep_s[1:] = sk[1:] != sk[:-1]
    keep = np.zeros(len(key), dtype=bool)
    keep[order] = keep_s
    urow = row[keep]
    ucol = col[keep]
    cnt = np.bincount(urow, minlength=B)

    # slot layout [B, S]: row r's entries in slots 0..cnt-1, rest weight 0
    o = np.argsort(urow, kind="stable")
    r_s = urow[o]
    c_s = ucol[o]
    cstart = np.concatenate([[0], np.cumsum(cnt)]).astype(np.int64)
    pos = np.arange(len(r_s), dtype=np.int64) - cstart[r_s]
    cols_slot = np.zeros((B, S), dtype=np.int64)
    w_slot = np.zeros((B, S), dtype=np.float32)
    cols_slot[r_s, pos] = c_s
    w_slot[r_s, pos] = 1.0 / np.maximum(cnt, 1)[r_s]

    feats16 = features.astype(np.float16)
    Wt_full = np.ascontiguousarray(W.T).astype(np.float16)
    WT8 = np.ascontiguousarray(W.T).astype(ml_dtypes.float8_e4m3)

    in_maps = []
    for k in range(N_CORES):
        rows = slice(k * R, (k + 1) * R)
        # slot-major: entry (r, s) at position s*R + r, so each on-device
        # 512-entry chunk is one slot plane [E, R]
        cf = cols_slot[rows].T.reshape(-1)
        wf = w_slot[rows].T.reshape(-1).astype(np.float16)
        xg = feats16[cf]                                  # [EN, F] fp16
        xgT_k = np.ascontiguousarray(xg.T)
        # exact weighted-mean features per output row (same fp16 inputs)
        xbar = np.einsum("srf,sr->rf",
                         xg.astype(np.float32).reshape(S, R, F),
                         w_slot[rows].T)                  # [R, F]
        xbT_k = np.ascontiguousarray(xbar.T).astype(np.float16)
        lo, hi = k * UL, min((k + 1) * UL, U)
        xpart = np.zeros((UL, F), dtype=np.float32)
        xpart[:hi - lo] = features[lo:hi]
        xT8 = xpart.T.astype(ml_dtypes.float8_e4m3)
        xa = np.zeros((128, 2, AW), dtype=ml_dtypes.float8_e4m3)
        xa[:, 0, :E] = WT8[0:128]
        xa[:, 1, :E] = WT8[128:256]
        xa[:, 0, E:] = xT8[0:128]
        xa[:, 1, E:] = xT8[128:256]
        gb4 = np.stack([gamma, beta,
                        np.full(E, 1.0 / (hi - lo), np.float32),
                        np.zeros(E, np.float32)], axis=1)
        in_maps.append({
            "xA": np.ascontiguousarray(xa.reshape(128, 2 * AW)),
            "xgT": xgT_k,
            "xbT": xbT_k,
            "Wt": Wt_full,
            "gb": np.ascontiguousarray(gb4),
            "wrow": np.ascontiguousarray(np.broadcast_to(wf, (128, EN))),
        })
    return in_maps


def kernel(features, W, b, gamma, beta, row_idx, col_idx, B=4096):
    global LAST_RESULTS
    in_maps = _prep_inputs(features, W, gamma, beta, row_idx, col_idx)
    nc = _build()
    res = run_bass_kernel_spmd(nc, in_maps, list(range(N_CORES)), trace=TRACE)
    LAST_RESULTS = res
    out = np.concatenate(
        [np.asarray(res.results[c]["outT"]).astype(np.float32).T
         for c in range(N_CORES)],
        axis=0)
    return out


# revision 67
# speedup vs baseline: 1.7254x; 1.7254x over previous
"""Trainium2 Bass kernel for MeanAggregator GNN message passing.

Computation (see reference):
  h = tanh(BN_trainmode(features @ W.T + b)) ; out = row-mean over sampled
  neighbor set (deduped membership mask) of h rows.  The linear bias b
  cancels exactly inside train-mode BN (shift-invariant), so it is dropped.

Strategy (8 cores, SPMD), rev11 — gather-free, stats-decoupled tail:
  - Shard OUTPUT rows across cores (512 rows/core).  The host pre-gathers
    the feature rows for each (row, slot) entry: every output row gets
    S=18 slot-planes (17 samples + pad; pad slots carry weight 0); each
    core receives a dense [256, 9216] fp16 entry matrix, a [128, 9216]
    replicated weight matrix, and a [256, 512] weighted-mean feature
    matrix (xbar).
  - BN batch stats need the full table; only channel sums/sumsq are used,
    so the table shard + W ride in float8e4 packed per k-tile and each
    512-column chunk is a single DoubleRow matmul.  The CC AllGather of
    the [128,2] partials has a hard floor in this environment: the CC
    cores wake ~60-110us after kernel launch regardless of doorbell.
  - To keep that floor off the compute path, the tanh pipeline runs
    DURING the collective using per-core LOCAL shard stats (a0, b0), and
    the global stats enter only through an exact first-order correction:
      out = out0 + da*(mbar - Q1) + db*(1 - Q0)
    with out0 = sum_s w*y0, Q0 = sum_s w*y0^2, Q1 = sum_s w*y0^2*mm,
    mbar = W @ xbar (exact, since sum_s w*mm is linear), da = a - a0,
    db = b - b0.  Local stats are off by <2.5%, so the dropped
    second-order term contributes ~3e-4 end-to-end (measured).
    Post-collective work collapses to the scale/shift chain + four
    fused per-partition DVE ops + one output DMA.
  - Entries are laid out SLOT-MAJOR (entry (r,s) at column s*R+r), so
    each 512-entry chunk is one slot plane [E,R]: ACT drains y0 (and the
    raw mm copy) straight from PSUM; DVE forms the three products and
    folds each plane into flat fp16 accumulators — no strided segmented
    reductions and no work left on the collective's critical path except
    the per-partition scale chain and three fused correction ops.
  - Output is [128, 512] (channels x rows) fp16 per core; host
    transposes and converts.
"""

import sys

for _p in ("/opt/trn_rl_repo", "/root/.axon_site/_ro/trn_rl_repo"):
    if _p not in sys.path:
        sys.path.append(_p)

import ml_dtypes
import numpy as np

import concourse.bass as bass
import concourse.bacc as bacc
import concourse.tile as tile
import concourse.mybir as mybir
from concourse.bass_utils import run_bass_kernel_spmd

F32 = mybir.dt.float32
F16 = mybir.dt.float16
F8 = mybir.dt.float8e4
AF = mybir.ActivationFunctionType
OP = mybir.AluOpType
AX = mybir.AxisListType
PM = mybir.MatmulPerfMode

N_CORES = 8
U, F, E, B = 50000, 256, 128, 4096
S = 18                  # slot stride per output row (17 samples + 1 pad, even for DVE 2x)
UL = 6272               # per-core table rows for stats (49 * 128)
AW = E + UL             # fp8 pack width per k-tile: [W | table]
R = B // N_CORES        # 512 output rows per core
EN = R * S              # 8704 entries per core (= 17 * 512 exactly)
CH = 512                # entry / table chunk width (one PSUM bank)
BN_EPS = 1e-5

U_CHUNKS = [(i * CH, CH) for i in range(UL // CH)]
if UL % CH:
    U_CHUNKS.append((UL - UL % CH, UL % CH))
E_CHUNKS = [(i * CH, CH) for i in range(EN // CH)]
XA_PIECES = [(0, E + 1536), (E + 1536, 1536), (E + 3072, 1536),
             (E + 4608, 1664)]

_CACHE = {}
LAST_RESULTS = None
TRACE = False


def _build():
    if "nc" in _CACHE:
        return _CACHE["nc"]

    nc = bacc.Bacc("TRN2", target_bir_lowering=False, debug=False,
                   enable_asserts=False, num_devices=N_CORES)

    # ---- I/O ----
    xA = nc.dram_tensor("xA", [128, 2 * AW], F8, kind="ExternalInput")
    xgT = nc.dram_tensor("xgT", [F, EN], F16, kind="ExternalInput")
    xbT = nc.dram_tensor("xbT", [F, R], F16, kind="ExternalInput")
    Wt = nc.dram_tensor("Wt", [F, E], F16, kind="ExternalInput")
    gb = nc.dram_tensor("gb", [E, 4], F32, kind="ExternalInput")
    wrow = nc.dram_tensor("wrow", [128, EN], F16, kind="ExternalInput")
    outT = nc.dram_tensor("outT", [E, R], F16, kind="ExternalOutput")

    # ---- internal DRAM (stats exchange): ReduceScatter of my [128,2]
    # partial tiled 8x -> every rank's output shard IS the full sum ----
    ag_in = nc.dram_tensor("ag_in", [N_CORES * E, 2], F32)
    ag_out = nc.dram_tensor("ag_out", [E, 2], F32)

    RG = [list(range(N_CORES))]
    xA3 = xA.ap().rearrange("p (two m) -> p two m", two=2)

    with tile.TileContext(nc) as tc:
        with (
            tc.tile_pool(name="const", bufs=1) as cpool,
            tc.tile_pool(name="rot", bufs=3) as rot,
        ):
            # ---- stats-critical load first: fp8 [W | table] piece 0 ----
            xa = cpool.tile([128, 2, AW], F8, tag="xa")
            p0, pn = XA_PIECES[0]
            nc.sync.dma_start(xa[:, :, p0:p0 + pn], xA3[:, :, p0:p0 + pn])

            wt0 = cpool.tile([128, E], F16, tag="wt0")
            wt1 = cpool.tile([128, E], F16, tag="wt1")
            nc.sync.dma_start(wt0[:], Wt[0:128, :])
            nc.sync.dma_start(wt1[:], Wt[128:256, :])
            xb0 = cpool.tile([128, R], F16, tag="xb0")
            xb1 = cpool.tile([128, R], F16, tag="xb1")
            nc.sync.dma_start(xb0[:], xbT[0:128, :])
            nc.sync.dma_start(xb1[:], xbT[128:256, :])
            gbt = cpool.tile([E, 4], F32, tag="gbt")
            nc.sync.dma_start(gbt[:], gb[:])
            epscol = cpool.tile([E, 1], F32, tag="epscol")
            nc.vector.memset(epscol[:], BN_EPS)

            # remaining table pieces, then the entry tensors (the latter
            # land inside the CC window; the quiesce barrier of the
            # collective covers only DMAs issued before it, so the entry
            # loads are issued after the collective below)
            for p0, pn in XA_PIECES[1:]:
                nc.sync.dma_start(xa[:, :, p0:p0 + pn], xA3[:, :, p0:p0 + pn])

            n_ch = len(U_CHUNKS)
            musum = cpool.tile([E, n_ch], F32, tag="musum")
            ssq = cpool.tile([E, n_ch], F32, tag="ssq")
            mbar = cpool.tile([E, R], F32, tag="mbar")

            # ---- phase A: fp8 DoubleRow table GEMM -> sum / sumsq;
            # then the exact linear-aggregate GEMM mbar = W @ xbar ----
            with tc.tile_pool(name="psA", bufs=1, space="PSUM") as psA:
                for ci, (u0, un) in enumerate(U_CHUNKS):
                    ps = psA.tile([128, un], F32, tag=f"ps{ci % 4}")
                    nc.tensor.matmul(
                        ps[:], xa[:, :, 0:E], xa[:, :, E + u0:E + u0 + un],
                        start=True, stop=True, perf_mode=PM.DoubleRow)
                    nc.vector.tensor_reduce(musum[:, ci:ci + 1], ps[:],
                                            axis=AX.X, op=OP.add)
                    sqd = rot.tile([128, un], F16, tag="sqd")
                    nc.scalar.activation(sqd[:], ps[:], AF.Square,
                                         accum_out=ssq[:, ci:ci + 1])
                psm = psA.tile([128, R], F32, tag="psm")
                nc.tensor.matmul(psm[:], wt0[:], xb0[:],
                                 start=True, stop=False)
                nc.tensor.matmul(psm[:], wt1[:], xb1[:],
                                 start=False, stop=True)
                nc.vector.tensor_copy(mbar[:], psm[:])

            # ---- my stats partial (for the collective + local stats) ----
            stats_sb = cpool.tile([E, 2], F32, tag="stats_sb")
            nc.vector.tensor_reduce(stats_sb[:, 0:1], musum[:], axis=AX.X,
                                    op=OP.add)
            nc.vector.tensor_reduce(stats_sb[:, 1:2], ssq[:], axis=AX.X,
                                    op=OP.add)
            for k in range(N_CORES):
                nc.gpsimd.dma_start(ag_in[k * E:(k + 1) * E, :],
                                    stats_sb[:])

            # ---- LOCAL shard stats -> a0, b0 (per-partition columns) ----
            invn = gbt[:, 2:3]
            mu0 = cpool.tile([E, 1], F32, tag="mu0")
            nc.vector.tensor_tensor(mu0[:], stats_sb[:, 0:1], invn,
                                    op=OP.mult)
            ex20 = cpool.tile([E, 1], F32, tag="ex20")
            nc.vector.tensor_tensor(ex20[:], stats_sb[:, 1:2], invn,
                                    op=OP.mult)
            var0 = cpool.tile([E, 1], F32, tag="var0")
            musq0 = cpool.tile([E, 1], F32, tag="musq0")
            nc.vector.tensor_tensor(musq0[:], mu0[:], mu0[:], op=OP.mult)
            nc.vector.tensor_tensor(var0[:], ex20[:], musq0[:],
                                    op=OP.subtract)
            sd0 = cpool.tile([E, 1], F32, tag="sd0")
            nc.scalar.activation(sd0[:], var0[:], AF.Sqrt,
                                 bias=epscol[:, 0:1])
            rinv0 = cpool.tile([E, 1], F32, tag="rinv0")
            nc.vector.reciprocal(rinv0[:], sd0[:])
            a0 = cpool.tile([E, 1], F32, tag="a0")
            nc.vector.tensor_tensor(a0[:], rinv0[:], gbt[:, 0:1], op=OP.mult)
            msc0 = cpool.tile([E, 1], F32, tag="msc0")
            nc.vector.tensor_tensor(msc0[:], mu0[:], a0[:], op=OP.mult)
            b0 = cpool.tile([E, 1], F32, tag="b0")
            nc.vector.tensor_tensor(b0[:], gbt[:, 1:2], msc0[:],
                                    op=OP.subtract)

            # entry features + replicated weights interleaved in quarter
            # pieces so GEMM chunk 0 (needs BOTH xg halves) and the first
            # yw product (needs wmt) start as soon as possible; all
            # stream during the CC window
            wmt = cpool.tile([128, EN], F16, tag="wmt")
            xg0 = cpool.tile([128, EN], F16, tag="xg0")
            xg1 = cpool.tile([128, EN], F16, tag="xg1")
            PW = EN // 4
            for p in range(4):
                sl = slice(p * PW, (p + 1) * PW)
                nc.sync.dma_start(xg0[:, sl], xgT[0:128, sl])
                nc.sync.dma_start(xg1[:, sl], xgT[128:256, sl])
                nc.sync.dma_start(wmt[:, sl], wrow[:, sl])

            # ---- phase B inside the CC window: per 512-entry chunk
            # GEMM -> { ACT y0 = tanh(a0*mm+b0), DVE raw copy }, then
            # GpSimd yw/y2w, DVE q, and per-64-row-block reduces ----
            # Entries are laid out SLOT-MAJOR (entry (r, s) at column
            # s*R + r), so each 512-entry chunk is exactly one slot
            # plane [E, R] and the three maps are flat fp16 accumulator
            # adds instead of strided segmented reductions.
            out0 = cpool.tile([E, R], F16, tag="out0")
            q0m = cpool.tile([E, R], F16, tag="q0m")
            q1m = cpool.tile([E, R], F16, tag="q1m")

            with tc.tile_pool(name="psB", bufs=1, space="PSUM") as psB:
                for ci, (e0, en) in enumerate(E_CHUNKS):
                    sl = slice(e0, e0 + en)
                    ps = psB.tile([128, en], F32, tag=f"pb{ci % 4}")
                    nc.tensor.matmul(ps[:], wt0[:], xg0[:, sl],
                                     start=True, stop=False)
                    nc.tensor.matmul(ps[:], wt1[:], xg1[:, sl],
                                     start=False, stop=True)
                    y0 = rot.tile([128, en], F16, tag="y0")
                    nc.scalar.activation(y0[:], ps[:], AF.Tanh,
                                         bias=b0[:, 0:1], scale=a0[:, 0:1])
                    mm = rot.tile([128, en], F16, tag="mm")
                    nc.scalar.copy(mm[:], ps[:])
                    if ci == 0:
                        yw, y2w, q = out0, q0m, q1m
                        nc.vector.tensor_tensor(yw[:], y0[:], wmt[:, sl],
                                                op=OP.mult)
                        nc.vector.tensor_tensor(y2w[:], y0[:], yw[:],
                                                op=OP.mult)
                        nc.vector.tensor_tensor(q[:], y2w[:], mm[:],
                                                op=OP.mult)
                    else:
                        yw = rot.tile([128, en], F16, tag="yw")
                        nc.vector.tensor_tensor(yw[:], y0[:], wmt[:, sl],
                                                op=OP.mult)
                        y2w = rot.tile([128, en], F16, tag="y2w")
                        nc.vector.tensor_tensor(y2w[:], y0[:], yw[:],
                                                op=OP.mult)
                        q = rot.tile([128, en], F16, tag="q")
                        nc.vector.tensor_tensor(q[:], y2w[:], mm[:],
                                                op=OP.mult)
                        with nc.allow_low_precision("fp16 plane accums"):
                            nc.vector.tensor_tensor(out0[:], out0[:], yw[:],
                                                    op=OP.add)
                            nc.vector.tensor_tensor(q0m[:], q0m[:], y2w[:],
                                                    op=OP.add)
                            nc.vector.tensor_tensor(q1m[:], q1m[:], q[:],
                                                    op=OP.add)
            # preload the SQRT table while the collective is in flight
            dum = cpool.tile([E, 1], F16, tag="dum")
            nc.scalar.activation(dum[:], epscol[:], AF.Sqrt)

            # ---- the collective (issued after the gpsimd products so
            # its completion drain does not block them) ----
            nc.gpsimd.collective_compute(
                "ReduceScatter", OP.add, replica_groups=RG,
                ins=[ag_in.ap()], outs=[ag_out.ap()])

            # ---- CC result (already summed by CCE) -> correction ----
            # The scheduler's sim models the collective as near-instant
            # and would order the CC-gated chain BEFORE the last plane
            # accumulates on the vector queue, dragging them behind the
            # real collective.  A dummy copy q1m->stats_g pins the recv
            # DMA (WAW) and everything after it behind the accumulators.
            stats_g = cpool.tile([E, 2], F32, tag="stats_g")
            nc.vector.tensor_copy(stats_g[0:1, 0:1], q1m[0:1, R - 1:R])
            nc.sync.dma_start(stats_g[:], ag_out.ap())

            mu = cpool.tile([E, 1], F32, tag="mu")
            nc.vector.tensor_scalar_mul(mu[:], stats_g[:, 0:1], 1.0 / U)
            ex2 = cpool.tile([E, 1], F32, tag="ex2")
            nc.vector.tensor_scalar_mul(ex2[:], stats_g[:, 1:2], 1.0 / U)
            # varneg = mu^2 - ex2 = -var; sd = sqrt(-varneg + eps)
            varneg = cpool.tile([E, 1], F32, tag="varneg")
            nc.vector.scalar_tensor_tensor(varneg[:], mu[:], mu[:, 0:1],
                                           ex2[:], op0=OP.mult,
                                           op1=OP.subtract)
            sd = cpool.tile([E, 1], F32, tag="sd")
            nc.scalar.activation(sd[:], varneg[:], AF.Sqrt,
                                 bias=epscol[:, 0:1], scale=-1.0)
            rinv = cpool.tile([E, 1], F32, tag="rinv")
            nc.vector.reciprocal(rinv[:], sd[:])
            a_g = cpool.tile([E, 1], F32, tag="a_g")
            nc.vector.tensor_tensor(a_g[:], rinv[:], gbt[:, 0:1], op=OP.mult)
            # bneg = mu*a - beta = -b_global (single fused op)
            bneg = cpool.tile([E, 1], F32, tag="bneg")
            nc.vector.scalar_tensor_tensor(bneg[:], mu[:], a_g[:, 0:1],
                                           gbt[:, 1:2], op0=OP.mult,
                                           op1=OP.subtract)

            m_ = cpool.tile([E, 1], F32, tag="m_")     # a0 - a = -da
            nc.vector.tensor_tensor(m_[:], a0[:], a_g[:], op=OP.subtract)
            db = cpool.tile([E, 1], F32, tag="db")     # b - b0
            nc.vector.scalar_tensor_tensor(db[:], bneg[:], -1.0, b0[:],
                                           op0=OP.mult, op1=OP.subtract)
            n_ = cpool.tile([E, 1], F32, tag="n_")     # -db = b0 + bneg
            nc.vector.tensor_tensor(n_[:], b0[:], bneg[:], op=OP.add)

            # out = out0 + da*(mbar - Q1) + db*(1 - Q0), via sign-carry:
            #   s1 = m*Q1 + out0 ; t1 = m*mbar - s1 ; t2 = n*Q0 - t1
            #   out = t2 + db   — split column-wise across DVE and the
            # (post-collective idle) GpSimd engine
            s1 = cpool.tile([E, R], F16, tag="s1")
            t1 = cpool.tile([E, R], F16, tag="t1")
            t2 = cpool.tile([E, R], F16, tag="t2")
            outsb = cpool.tile([E, R], F16, tag="outsb")
            nc.vector.scalar_tensor_tensor(s1[:], q1m[:], m_[:, 0:1],
                                           out0[:], op0=OP.mult, op1=OP.add)
            nc.vector.scalar_tensor_tensor(t1[:], mbar[:], m_[:, 0:1],
                                           s1[:], op0=OP.mult,
                                           op1=OP.subtract)
            nc.vector.scalar_tensor_tensor(t2[:], q0m[:], n_[:, 0:1],
                                           t1[:], op0=OP.mult,
                                           op1=OP.subtract)
            nc.vector.tensor_scalar_add(outsb[:], t2[:], db[:, 0:1])

            nc.sync.dma_start(outT.ap(), outsb[:])

    nc.compile()
    _CACHE["nc"] = nc
    return nc


def _prep_inputs(features, W, gamma, beta, row_idx, col_idx):
    """Host-side sharding: dedup mask entries, lay out 17 slots per output
    row (zero-weight padding), pre-gather entry feature rows per core."""
    features = np.asarray(features, dtype=np.float32)
    W = np.asarray(W, dtype=np.float32)
    gamma = np.asarray(gamma, dtype=np.float32)
    beta = np.asarray(beta, dtype=np.float32)
    row = np.asarray(row_idx).astype(np.int64)
    col = np.asarray(col_idx).astype(np.int64)

    # dedup (row, col) pairs: mask "set" semantics
    key = row * np.int64(U) + col
    order = np.argsort(key, kind="stable")
    sk = key[order]
    keep_s = np.ones(len(sk), dtype=bool)
    keep_s[1:] = sk[1:] != sk[:-1]
    keep = np.zeros(len(key), dtype=bool)
    keep[order] = keep_s
    urow = row[keep]
    ucol = col[keep]
    cnt = np.bincount(urow, minlength=B)

    # slot layout [B, S]: row r's entries in slots 0..cnt-1, rest weight 0
    o = np.argsort(urow, kind="stable")
    r_s = urow[o]
    c_s = ucol[o]
    cstart = np.concatenate([[0], np.cumsum(cnt)]).astype(np.int64)
    pos = np.arange(len(r_s), dtype=np.int64) - cstart[r_s]
    cols_slot = np.zeros((B, S), dtype=np.int64)
    w_slot = np.zeros((B, S), dtype=np.float32)
    cols_slot[r_s, pos] = c_s
    w_slot[r_s, pos] = 1.0 / np.maximum(cnt, 1)[r_s]

    feats16 = features.astype(np.float16)
    Wt_full = np.ascontiguousarray(W.T).astype(np.float16)
    WT8 = np.ascontiguousarray(W.T).astype(ml_dtypes.float8_e4m3)

    in_maps = []
    for k in range(N_CORES):
        rows = slice(k * R, (k + 1) * R)
        # slot-major: entry (r, s) at position s*R + r, so each on-device
        # 512-entry chunk is one slot plane [E, R]
        cf = cols_slot[rows].T.reshape(-1)
        wf = w_slot[rows].T.reshape(-1).astype(np.float16)
        xg = feats16[cf]                                  # [EN, F] fp16
        xgT_k = np.ascontiguousarray(xg.T)
        # exact weighted-mean features per output row (same fp16 inputs)
        xbar = np.einsum("srf,sr->rf",
                         xg.astype(np.float32).reshape(S, R, F),
                         w_slot[rows].T)                  # [R, F]
        xbT_k = np.ascontiguousarray(xbar.T).astype(np.float16)
        lo, hi = k * UL, min((k + 1) * UL, U)
        xpart = np.zeros((UL, F), dtype=np.float32)
        xpart[:hi - lo] = features[lo:hi]
        xT8 = xpart.T.astype(ml_dtypes.float8_e4m3)
        xa = np.zeros((128, 2, AW), dtype=ml_dtypes.float8_e4m3)
        xa[:, 0, :E] = WT8[0:128]
        xa[:, 1, :E] = WT8[128:256]
        xa[:, 0, E:] = xT8[0:128]
        xa[:, 1, E:] = xT8[128:256]
        gb4 = np.stack([gamma, beta,
                        np.full(E, 1.0 / (hi - lo), np.float32),
                        np.zeros(E, np.float32)], axis=1)
        in_maps.append({
            "xA": np.ascontiguousarray(xa.reshape(128, 2 * AW)),
            "xgT": xgT_k,
            "xbT": xbT_k,
            "Wt": Wt_full,
            "gb": np.ascontiguousarray(gb4),
            "wrow": np.ascontiguousarray(np.broadcast_to(wf, (128, EN))),
        })
    return in_maps


def kernel(features, W, b, gamma, beta, row_idx, col_idx, B=4096):
    global LAST_RESULTS
    in_maps = _prep_inputs(features, W, gamma, beta, row_idx, col_idx)
    nc = _build()
    res = run_bass_kernel_spmd(nc, in_maps, list(range(N_CORES)), trace=TRACE)
    LAST_RESULTS = res
    out = np.concatenate(
        [np.asarray(res.results[c]["outT"]).astype(np.float32).T
         for c in range(N_CORES)],
        axis=0)
    return out
